# revision 6
# baseline (speedup 1.0000x reference)
"""Trainium2 Bass kernel for ContextQueryAtt (BiDAF-style context-query attention).

v2 redesign vs baseline: all matmuls in float32r (ctx/query cast to f32r by
the load DMAs), f32r PE transposes (1.5 cyc/row vs 2.0), C2 matmul full-rate,
rowsums via one DVE tensor_reduce instead of 8 ACT accum copies, merged
PSUM->SBUF copies, and ACT/DVE/Pool load balancing of copies and elementwise
muls. Loads ride the SWDGE (gpsimd) ring; stores split across the SP and ACT
HWDGE rings.

Math (per batch b):
    sim[c,q] = ctx[c,:]@Wc + q[q,:]@Wq + (ctx[c,:]*Wcq)@q[q,:] + bias
    S1 = softmax_q(sim), S2 = softmax_c(sim)
    A  = S1 @ query
    B  = (S1 @ S2^T) @ ctx  ==  S1 @ (S2^T @ ctx)
    out = concat([ctx, A, ctx*A, ctx*B], axis=-1)

Softmax without max-subtraction (|sim| <~ 15, exp safe in fp32); the
normalizations are postponed into per-partition scales on the PSUM->SBUF
copies (1/rowsum for A/B, 1/colsum for C2 = S2^T@ctx).

Data-parallel over batch: 4 batches per core x 8 cores, identical program.
Masks are always all-ones for this problem; exact numpy fallback otherwise.
"""

import sys

if "/opt/trn_rl_repo" not in sys.path:
    sys.path.insert(0, "/opt/trn_rl_repo")

from contextlib import ExitStack

import numpy as np

import os

import concourse.bacc as bacc
import concourse.masks as cmasks
import concourse.mybir as mybir
import concourse.tile as tile
from concourse.bass_utils import run_bass_kernel_spmd

N_CORES = 8
BS, C, Q, D = 32, 1024, 128, 512
BPC = BS // N_CORES      # batches per core
CT = C // 128            # context tiles (8)
DT = D // 128            # d tiles (4)
F32 = mybir.dt.float32
F32R = mybir.dt.float32r
AF = mybir.ActivationFunctionType
AX = mybir.AxisListType
ALU = mybir.AluOpType


def build_program(bias_f: float, repeat: int = 1):
    n_pool_muls = int(os.environ.get("K_POOLMULS", "4"))   # of 16 ctx*A/B muls
    stage_bufs = int(os.environ.get("K_STAGEBUFS", "2"))   # staging double/triple buffer
    st_gran = int(os.environ.get("K_STGRAN", "2"))         # c-tiles per stage store
    body_reps = int(os.environ.get("K_BODYREPS", "1"))     # batch-loop copies per For_i body
    dma_only = os.environ.get("K_DMAONLY", "0") == "1"     # IO-roofline ablation
    cast_loads = os.environ.get("K_CASTLOAD", "1") == "1"  # f32r cast in DMA
    nc = bacc.Bacc("TRN2", target_bir_lowering=False, debug=False,
                   num_devices=N_CORES)

    ctx_d = nc.dram_tensor("context", [BPC, C, D], F32, kind="ExternalInput")
    q_d = nc.dram_tensor("query", [BPC, Q, D], F32, kind="ExternalInput")
    w_d = nc.dram_tensor("wpack", [128, 3 * DT], F32, kind="ExternalInput")
    out_d = nc.dram_tensor("out", [BPC, C, 4 * D], F32, kind="ExternalOutput")

    with tile.TileContext(nc) as tc, ExitStack() as ctx:
        # ---- constants ----
        cpool = ctx.enter_context(tc.tile_pool(name="const", bufs=1))
        ident = cpool.tile([128, 128], F32, tag="ident")
        cmasks.make_identity(nc, ident[:])
        ident_r = cpool.tile([128, 128], F32R, tag="identr")
        nc.scalar.copy(ident_r[:], ident[:])
        ones_f = cpool.tile([1, 128], F32, tag="onesf")
        nc.vector.memset(ones_f[:], 1.0)
        ones_row = cpool.tile([1, 128], F32R, tag="ones")
        nc.scalar.copy(ones_row[:], ones_f[:])
        wpack = cpool.tile([128, 3 * DT], F32, tag="wpack")
        nc.sync.dma_start(wpack[:], w_d.ap())
        wpack_r = cpool.tile([128, 3 * DT], F32R, tag="wpackr")
        nc.gpsimd.dma_start(wpack_r[:], w_d.ap())   # casting DMA -> f32r

        # ---- SBUF pools ----
        p_ctx = ctx.enter_context(tc.tile_pool(name="ctx", bufs=2))
        p_q = ctx.enter_context(tc.tile_pool(name="q", bufs=2))
        p_qt = ctx.enter_context(tc.tile_pool(name="qt", bufs=2))
        p_ctxt = ctx.enter_context(tc.tile_pool(name="ctxt", bufs=2))
        p_et = ctx.enter_context(tc.tile_pool(name="et", bufs=2))
        p_e = ctx.enter_context(tc.tile_pool(name="e", bufs=2))
        p_c2 = ctx.enter_context(tc.tile_pool(name="c2", bufs=2))
        p_stage = ctx.enter_context(tc.tile_pool(name="stage", bufs=stage_bufs))
        p_small = ctx.enter_context(tc.tile_pool(name="small", bufs=2))
        p_csim = ctx.enter_context(tc.tile_pool(name="csim", bufs=2))

        # ---- PSUM pools (8 banks: 2 tp + 2 sim + 4 ab) ----
        ps_tp = ctx.enter_context(tc.tile_pool(name="ps_tp", bufs=2, space="PSUM"))
        ps_sim = ctx.enter_context(tc.tile_pool(name="ps_sim", bufs=2, space="PSUM"))
        ps_ab = ctx.enter_context(tc.tile_pool(name="ps_ab", bufs=2, space="PSUM"))

        import contextlib
        rep_ctx = tc.For_i(0, repeat, 1) if repeat > 1 else contextlib.nullcontext()
        with rep_ctx:
          for b in [bb % BPC for bb in range(BPC * body_reps)]:
            ctx_v = ctx_d.ap()[b].rearrange("(t p) d -> p t d", p=128)
            out_v = out_d.ap()[b].rearrange("(t p) e -> p t e", p=128)

            # ---- loads (SWDGE ring; own ring, never behind stores) ----
            if cast_loads:
                # cast fp32 -> f32r inside the DMA
                ctx_r = p_ctx.tile([128, CT, D], F32R, tag="ctx")
                nc.gpsimd.dma_start(ctx_r[:, 0:4, :], ctx_v[:, 0:4, :])
                nc.gpsimd.dma_start(ctx_r[:, 4:8, :], ctx_v[:, 4:8, :])
                q_r = p_q.tile([128, D], F32R, tag="q")
                nc.gpsimd.dma_start(q_r[:], q_d.ap()[b])
            else:
                # plain loads; cast on-chip on Pool
                ctx_f = p_ctx.tile([128, CT, D], F32, tag="ctxf")
                nc.gpsimd.dma_start(ctx_f[:, 0:4, :], ctx_v[:, 0:4, :])
                nc.gpsimd.dma_start(ctx_f[:, 4:8, :], ctx_v[:, 4:8, :])
                q_f = p_q.tile([128, D], F32, tag="qf")
                nc.gpsimd.dma_start(q_f[:], q_d.ap()[b])
                if not dma_only:
                    ctx_r = p_ctx.tile([128, CT, D], F32R, tag="ctx")
                    for ct in range(CT):
                        nc.gpsimd.tensor_copy(ctx_r[:, ct, :], ctx_f[:, ct, :])
                    q_r = p_q.tile([128, D], F32R, tag="q")
                    nc.gpsimd.tensor_copy(q_r[:], q_f[:])
                else:
                    ctx_r = ctx_f.bitcast(F32R)

            if dma_only:
                # IO roofline: same loads + same store shapes, no compute.
                nc.sync.dma_start(out_v[:, :, 0:D], ctx_r[:].bitcast(F32))
                for g in range(2):
                    stage = p_stage.tile([128, 4, 4, 512], F32, tag="stage")
                    nc.vector.memset(stage[:, 0, 0, 0:4], 0.0)
                    st_eng = nc.sync if g == 0 else nc.scalar
                    for h in range(2):
                        st_eng.dma_start(
                            out_v[:, g * 4 + h * 2:g * 4 + (h + 1) * 2, D:4 * D],
                            stage[:, h * 2:(h + 1) * 2, 0:3, :])
                continue

            # context passthrough: data ready at batch start; issue early so
            # the DMA engines have work while this batch's compute runs.
            nc.sync.dma_start(out_v[:, :, 0:D], ctx_r[:].bitcast(F32))

            # ---- query transposes: qt (plain q^T), qwt (q^T * Wcq) ----
            ps_q = ps_tp.tile([128, 512], F32R, tag="tp")
            for t in range(DT):
                nc.tensor.transpose(
                    ps_q[:, t * 128:(t + 1) * 128],
                    q_r[:, t * 128:(t + 1) * 128], ident_r[:])
            qt_sb = p_qt.tile([128, DT * 128], F32R, tag="qt")
            nc.scalar.copy(qt_sb[:], ps_q[:])
            qwt_sb = p_qt.tile([128, DT * 128], F32R, tag="qwt")
            for t in range(DT):
                nc.scalar.activation(
                    qwt_sb[:, t * 128:(t + 1) * 128],
                    ps_q[:, t * 128:(t + 1) * 128],
                    AF.Copy, scale=wpack[:, 2 * DT + t:2 * DT + t + 1])

            # ---- context transposes: ctxt[d-part][dt, c] (f32r) ----
            ctxt_sb = p_ctxt.tile([128, DT, C], F32R, tag="ctxt")
            for t in range(DT):
                for g in range(2):
                    ps_c = ps_tp.tile([128, 512], F32R, tag="tp")
                    for i in range(4):
                        ct = g * 4 + i
                        nc.tensor.transpose(
                            ps_c[:, i * 128:(i + 1) * 128],
                            ctx_r[:, ct, t * 128:(t + 1) * 128], ident_r[:])
                    eng = nc.scalar if t < 2 else nc.vector
                    if eng is nc.scalar:
                        eng.copy(ctxt_sb[:, t, g * 512:(g + 1) * 512], ps_c[:])
                    else:
                        eng.tensor_copy(ctxt_sb[:, t, g * 512:(g + 1) * 512],
                                        ps_c[:])

            # ---- q_sim column via PE (N=2, col 1 junk) ----
            ps_qs = ps_sim.tile([128, 512], F32, tag="sim")
            for t in range(DT):
                nc.tensor.matmul(
                    ps_qs[:, 0:2],
                    qt_sb[:, t * 128:(t + 1) * 128],
                    wpack_r[:, t:t + 2],
                    start=(t == 0), stop=(t == DT - 1))
            bias_col = p_small.tile([128, 1], F32, tag="biascol")
            nc.vector.tensor_scalar_add(bias_col[:], ps_qs[:, 0:1], bias_f)

            # ---- c_sim^T[1, c] = Wc^T @ ctxt (lhsT M=2, row 1 junk) ----
            csim_sb = p_csim.tile([1, C], F32R, tag="csim")
            for g in range(2):
                ps_cs = ps_tp.tile([2, 512], F32, tag="tp")
                for t in range(DT):
                    nc.tensor.matmul(
                        ps_cs[:],
                        wpack_r[:, DT + t:DT + t + 2],
                        ctxt_sb[:, t, g * 512:(g + 1) * 512],
                        start=(t == 0), stop=(t == DT - 1))
                nc.scalar.copy(csim_sb[:, g * 512:(g + 1) * 512],
                               ps_cs[0:1, :])

            # ---- sim^T = qwt^T @ ctxt + ones^T @ csim; E^T = exp(+qsim) ----
            et_sb = p_et.tile([128, C], F32R, tag="et")
            cs_parts = p_small.tile([128, 2], F32, tag="csparts")
            for g in range(2):
                ps_s = ps_sim.tile([128, 512], F32, tag="sim")
                for t in range(DT):
                    nc.tensor.matmul(
                        ps_s[:],
                        qwt_sb[:, t * 128:(t + 1) * 128],
                        ctxt_sb[:, t, g * 512:(g + 1) * 512],
                        start=(t == 0), stop=False)
                nc.tensor.matmul(
                    ps_s[:], ones_row[:],
                    csim_sb[:, g * 512:(g + 1) * 512],
                    start=False, stop=True)
                nc.scalar.activation(
                    et_sb[:, g * 512:(g + 1) * 512], ps_s[:],
                    AF.Exp, bias=bias_col[:],
                    accum_out=cs_parts[:, g:g + 1])

            cs_col = p_small.tile([128, 1], F32, tag="cscol")
            nc.vector.tensor_add(cs_col[:], cs_parts[:, 0:1], cs_parts[:, 1:2])
            rcs_col = p_small.tile([128, 1], F32, tag="rcscol")
            nc.vector.reciprocal(rcs_col[:], cs_col[:])

            # ---- E tiles [c-part, ct, q] via PE transpose (f32r) ----
            e_sb = p_e.tile([128, CT, 128], F32R, tag="e")
            for g in range(2):
                ps_e = ps_tp.tile([128, 512], F32R, tag="tp")
                for i in range(4):
                    ct = g * 4 + i
                    nc.tensor.transpose(
                        ps_e[:, i * 128:(i + 1) * 128],
                        et_sb[:, ct * 128:(ct + 1) * 128],
                        ident_r[:])
                nc.scalar.copy(
                    e_sb[:, g * 4:(g + 1) * 4, :].rearrange("p t q -> p (t q)"),
                    ps_e[:])

            # rowsums rs[c] = sum_q E[c,q] in one DVE reduce; rrs = 1/rs
            rs_sb = p_small.tile([128, CT], F32, tag="rs")
            nc.vector.tensor_reduce(rs_sb[:], e_sb[:], axis=AX.X, op=ALU.add)
            rrs_sb = p_small.tile([128, CT], F32, tag="rrs")
            nc.vector.reciprocal(rrs_sb[:], rs_sb[:])

            # ---- C2 = S2^T @ ctx = (E^T-sums) / cs   (f32r full rate) ----
            ps_c2 = ps_sim.tile([128, 512], F32, tag="sim")
            for ct in range(CT):
                nc.tensor.matmul(
                    ps_c2[:],
                    e_sb[:, ct, :],
                    ctx_r[:, ct, :],
                    start=(ct == 0), stop=(ct == CT - 1))
            c2_sb = p_c2.tile([128, D], F32R, tag="c2")
            nc.scalar.activation(c2_sb[:], ps_c2[:], AF.Copy, scale=rcs_col[:])

            # ---- per c-tile: A, B, ctx*A, ctx*B into staging; DMA out ----
            # stage quadrants per tile: [A | CA | CB | B]; only 0:3 stored.
            n_pool = 0
            for g in range(2):
                stage = p_stage.tile([128, 4, 4, 512], F32, tag="stage")
                for i in range(4):
                    ct = g * 4 + i
                    ps = ps_ab.tile([128, 1024], F32, tag="ab")
                    nc.tensor.matmul(
                        ps[:, 0:512],
                        et_sb[:, ct * 128:(ct + 1) * 128],
                        q_r[:], start=True, stop=True)
                    nc.tensor.matmul(
                        ps[:, 512:1024],
                        et_sb[:, ct * 128:(ct + 1) * 128],
                        c2_sb[:], start=True, stop=True)
                    # A = ps[:, 0:512]/rs on ACT; B = ps[:, 512:1024]/rs on DVE
                    nc.scalar.activation(
                        stage[:, i, 0, :], ps[:, 0:512], AF.Copy,
                        scale=rrs_sb[:, ct:ct + 1])
                    nc.vector.tensor_scalar_mul(
                        stage[:, i, 3, :], ps[:, 512:1024],
                        rrs_sb[:, ct:ct + 1])
                    # CA = ctx*A ; CB = ctx*B
                    ctx_ct = ctx_r[:, ct, :].bitcast(F32)
                    if n_pool < n_pool_muls:
                        n_pool += 1
                        nc.gpsimd.tensor_mul(
                            stage[:, i, 2, :], ctx_ct, stage[:, i, 3, :])
                    else:
                        nc.vector.tensor_mul(
                            stage[:, i, 2, :], ctx_ct, stage[:, i, 3, :])
                    nc.vector.tensor_mul(
                        stage[:, i, 1, :], ctx_ct, stage[:, i, 0, :])
                    if (i + 1) % st_gran == 0:
                        # store per st_gran c-tiles: earlier issue, smoother DMA
                        st_eng = nc.sync if g == 0 else nc.scalar
                        i0 = i + 1 - st_gran
                        st_eng.dma_start(
                            out_v[:, g * 4 + i0:g * 4 + i + 1, D:4 * D],
                            stage[:, i0:i + 1, 0:3, :])

    nc.compile()
    return nc


def _numpy_reference(context, query, c_mask, q_mask, Wq, Wc, Wcq, bias):
    """Exact fallback (matches reference.py) for non-all-ones masks."""
    NEG = -1e30
    q_sim = (query @ Wq[:, 0])[:, None, :]
    c_sim = (context @ Wc[:, 0])[:, :, None]
    cq_sim = np.einsum("bcd,bqd->bcq", context * Wcq, query)
    sim = q_sim + c_sim + cq_sim + bias
    qm = q_mask[:, None, :]
    cm = c_mask[:, :, None]
    q_logits = sim * qm + (1.0 - qm) * NEG
    c_logits = sim * cm + (1.0 - cm) * NEG

    def softmax(x, axis):
        x = x - x.max(axis=axis, keepdims=True)
        e = np.exp(x)
        return e / e.sum(axis=axis, keepdims=True)

    S1 = softmax(q_logits, -1)
    S2 = softmax(c_logits, 1)
    A = np.einsum("bcq,bqd->bcd", S1, query)
    B = np.einsum("bcq,bqd->bcd", S1, np.einsum("bkq,bkd->bqd", S2, context))
    return np.concatenate([context, A, context * A, context * B],
                          axis=2).astype(np.float32)


def make_in_maps(inputs) -> list:
    context = np.ascontiguousarray(np.asarray(inputs["context"], dtype=np.float32))
    query = np.ascontiguousarray(np.asarray(inputs["query"], dtype=np.float32))
    Wq = np.asarray(inputs["Wq"], dtype=np.float32)
    Wc = np.asarray(inputs["Wc"], dtype=np.float32)
    Wcq = np.asarray(inputs["Wcq"], dtype=np.float32)

    # pack the tiny weight vectors as [128, DT] columns (d = t*128 + p)
    def cols(w):
        return np.ascontiguousarray(w.reshape(DT, 128).T.astype(np.float32))

    wpack = np.concatenate(
        [cols(Wq[:, 0]), cols(Wc[:, 0]), cols(Wcq.reshape(-1))], axis=1)

    in_maps = []
    for i in range(N_CORES):
        in_maps.append({
            "context": np.ascontiguousarray(context[i * BPC:(i + 1) * BPC]),
            "query": np.ascontiguousarray(query[i * BPC:(i + 1) * BPC]),
            "wpack": wpack,
        })
    return in_maps


def kernel(**inputs) -> np.ndarray:
    c_mask = np.asarray(inputs["c_mask"], dtype=np.float32)
    q_mask = np.asarray(inputs["q_mask"], dtype=np.float32)
    bias = np.asarray(inputs["bias"], dtype=np.float32)

    if not (np.all(c_mask == 1.0) and np.all(q_mask == 1.0)):
        return _numpy_reference(
            np.asarray(inputs["context"], dtype=np.float32),
            np.asarray(inputs["query"], dtype=np.float32),
            c_mask, q_mask,
            np.asarray(inputs["Wq"], dtype=np.float32),
            np.asarray(inputs["Wc"], dtype=np.float32),
            np.asarray(inputs["Wcq"], dtype=np.float32),
            float(bias.reshape(-1)[0]))

    nc = build_program(float(bias.reshape(-1)[0]))
    in_maps = make_in_maps(inputs)
    try:
        res = run_bass_kernel_spmd(nc, in_maps, core_ids=list(range(N_CORES)))
    except Exception:
        # transient device wedge (e.g. NRT_EXEC_UNIT_UNRECOVERABLE): one retry
        res = run_bass_kernel_spmd(nc, in_maps, core_ids=list(range(N_CORES)))
    global last_results
    last_results = res
    out = np.concatenate([res.results[i]["out"] for i in range(N_CORES)], axis=0)
    return out


last_results = None


# revision 7
# speedup vs baseline: 1.3641x; 1.3641x over previous
"""Trainium2 Bass kernel for ContextQueryAtt (BiDAF-style context-query attention).

v2 redesign vs baseline: all matmuls in float32r (ctx/query cast to f32r by
the load DMAs), f32r PE transposes (1.5 cyc/row vs 2.0), C2 matmul full-rate,
rowsums via one DVE tensor_reduce instead of 8 ACT accum copies, merged
PSUM->SBUF copies, and ACT/DVE/Pool load balancing of copies and elementwise
muls. Loads ride the SWDGE (gpsimd) ring; stores split across the SP and ACT
HWDGE rings.

Math (per batch b):
    sim[c,q] = ctx[c,:]@Wc + q[q,:]@Wq + (ctx[c,:]*Wcq)@q[q,:] + bias
    S1 = softmax_q(sim), S2 = softmax_c(sim)
    A  = S1 @ query
    B  = (S1 @ S2^T) @ ctx  ==  S1 @ (S2^T @ ctx)
    out = concat([ctx, A, ctx*A, ctx*B], axis=-1)

Softmax without max-subtraction (|sim| <~ 15, exp safe in fp32); the
normalizations are postponed into per-partition scales on the PSUM->SBUF
copies (1/rowsum for A/B, 1/colsum for C2 = S2^T@ctx).

Data-parallel over batch: 4 batches per core x 8 cores, identical program.
Masks are always all-ones for this problem; exact numpy fallback otherwise.
"""

import sys

if "/opt/trn_rl_repo" not in sys.path:
    sys.path.insert(0, "/opt/trn_rl_repo")

from contextlib import ExitStack

import numpy as np

import os

import concourse.bacc as bacc
import concourse.masks as cmasks
import concourse.mybir as mybir
import concourse.tile as tile
from concourse.bass_utils import run_bass_kernel_spmd

N_CORES = 8
BS, C, Q, D = 32, 1024, 128, 512
BPC = BS // N_CORES      # batches per core
CT = C // 128            # context tiles (8)
DT = D // 128            # d tiles (4)
F32 = mybir.dt.float32
F32R = mybir.dt.float32r
AF = mybir.ActivationFunctionType
AX = mybir.AxisListType
ALU = mybir.AluOpType


def build_program(bias_f: float, repeat: int = 1):
    n_pool_muls = int(os.environ.get("K_POOLMULS", "4"))   # of 16 ctx*A/B muls
    stage_bufs = int(os.environ.get("K_STAGEBUFS", "2"))   # staging double/triple buffer
    st_gran = int(os.environ.get("K_STGRAN", "2"))         # c-tiles per stage store
    body_reps = int(os.environ.get("K_BODYREPS", "1"))     # batch-loop copies per For_i body
    dma_only = os.environ.get("K_DMAONLY", "0") == "1"     # IO-roofline ablation
    cast_mode = os.environ.get("K_CASTLOAD", "1")  # 1: cast loads; 0: pool casts; 2: plain ctx loads + fp32 C2
    cast_loads = cast_mode == "1"
    nc = bacc.Bacc("TRN2", target_bir_lowering=False, debug=False,
                   num_devices=N_CORES)

    ctx_d = nc.dram_tensor("context", [BPC, C, D], F32, kind="ExternalInput")
    q_d = nc.dram_tensor("query", [BPC, Q, D], F32, kind="ExternalInput")
    w_d = nc.dram_tensor("wpack", [128, 3 * DT], F32, kind="ExternalInput")
    out_d = nc.dram_tensor("out", [BPC, C, 4 * D], F32, kind="ExternalOutput")

    with tile.TileContext(nc) as tc, ExitStack() as ctx:
        # ---- constants ----
        cpool = ctx.enter_context(tc.tile_pool(name="const", bufs=1))
        ident = cpool.tile([128, 128], F32, tag="ident")
        cmasks.make_identity(nc, ident[:])
        ident_r = cpool.tile([128, 128], F32R, tag="identr")
        nc.scalar.copy(ident_r[:], ident[:])
        ones_f = cpool.tile([1, 128], F32, tag="onesf")
        nc.vector.memset(ones_f[:], 1.0)
        ones_row = cpool.tile([1, 128], F32R, tag="ones")
        nc.scalar.copy(ones_row[:], ones_f[:])
        wpack = cpool.tile([128, 3 * DT], F32, tag="wpack")
        nc.sync.dma_start(wpack[:], w_d.ap())
        wpack_r = cpool.tile([128, 3 * DT], F32R, tag="wpackr")
        nc.gpsimd.dma_start(wpack_r[:], w_d.ap())   # casting DMA -> f32r

        # ---- SBUF pools ----
        p_ctx = ctx.enter_context(tc.tile_pool(name="ctx", bufs=2))
        p_q = ctx.enter_context(tc.tile_pool(name="q", bufs=2))
        p_qt = ctx.enter_context(tc.tile_pool(name="qt", bufs=2))
        p_ctxt = ctx.enter_context(tc.tile_pool(name="ctxt", bufs=2))
        p_et = ctx.enter_context(tc.tile_pool(name="et", bufs=2))
        p_e = ctx.enter_context(tc.tile_pool(name="e", bufs=2))
        p_c2 = ctx.enter_context(tc.tile_pool(name="c2", bufs=2))
        p_stage = ctx.enter_context(tc.tile_pool(name="stage", bufs=stage_bufs))
        p_small = ctx.enter_context(tc.tile_pool(name="small", bufs=2))
        p_csim = ctx.enter_context(tc.tile_pool(name="csim", bufs=2))

        # ---- PSUM pools (8 banks: 2 tp + 2 sim + 4 ab) ----
        ps_tp = ctx.enter_context(tc.tile_pool(name="ps_tp", bufs=2, space="PSUM"))
        ps_sim = ctx.enter_context(tc.tile_pool(name="ps_sim", bufs=2, space="PSUM"))
        ps_ab = ctx.enter_context(tc.tile_pool(name="ps_ab", bufs=2, space="PSUM"))

        import contextlib
        rep_ctx = tc.For_i(0, repeat, 1) if repeat > 1 else contextlib.nullcontext()
        with rep_ctx:
          for b in [bb % BPC for bb in range(BPC * body_reps)]:
            ctx_v = ctx_d.ap()[b].rearrange("(t p) d -> p t d", p=128)
            out_v = out_d.ap()[b].rearrange("(t p) e -> p t e", p=128)

            # ---- loads (SWDGE ring; own ring, never behind stores) ----
            if cast_mode == "2":
                # plain ctx loads (faster DMA); only query cast to f32r.
                # C2 then runs fp32 (PE has slack under the DMA floor).
                ctx_f = p_ctx.tile([128, CT, D], F32, tag="ctxf")
                nc.gpsimd.dma_start(ctx_f[:, 0:4, :], ctx_v[:, 0:4, :])
                nc.gpsimd.dma_start(ctx_f[:, 4:8, :], ctx_v[:, 4:8, :])
                q_r = p_q.tile([128, D], F32R, tag="q")
                nc.gpsimd.dma_start(q_r[:], q_d.ap()[b])
                ctx_r = None
            elif cast_loads:
                # cast fp32 -> f32r inside the DMA
                ctx_r = p_ctx.tile([128, CT, D], F32R, tag="ctx")
                nc.gpsimd.dma_start(ctx_r[:, 0:4, :], ctx_v[:, 0:4, :])
                nc.gpsimd.dma_start(ctx_r[:, 4:8, :], ctx_v[:, 4:8, :])
                q_r = p_q.tile([128, D], F32R, tag="q")
                nc.gpsimd.dma_start(q_r[:], q_d.ap()[b])
            else:
                # plain loads; cast on-chip on Pool
                ctx_f = p_ctx.tile([128, CT, D], F32, tag="ctxf")
                nc.gpsimd.dma_start(ctx_f[:, 0:4, :], ctx_v[:, 0:4, :])
                nc.gpsimd.dma_start(ctx_f[:, 4:8, :], ctx_v[:, 4:8, :])
                q_f = p_q.tile([128, D], F32, tag="qf")
                nc.gpsimd.dma_start(q_f[:], q_d.ap()[b])
                if not dma_only:
                    ctx_r = p_ctx.tile([128, CT, D], F32R, tag="ctx")
                    for ct in range(CT):
                        nc.gpsimd.tensor_copy(ctx_r[:, ct, :], ctx_f[:, ct, :])
                    q_r = p_q.tile([128, D], F32R, tag="q")
                    nc.gpsimd.tensor_copy(q_r[:], q_f[:])
                else:
                    ctx_r = ctx_f.bitcast(F32R)

            if cast_mode == "2":
                ctx_elem = ctx_f[:]           # fp32 view for muls/stores
                tp_ident, tp_dtype = ident, F32
                e_dtype = F32                 # e_sb feeds the fp32 C2
            else:
                ctx_elem = ctx_r[:].bitcast(F32)
                tp_ident, tp_dtype = ident_r, F32R
                e_dtype = F32R

            if dma_only:
                # IO roofline: same loads + same store shapes, no compute.
                nc.sync.dma_start(out_v[:, :, 0:D], ctx_elem)
                for g in range(2):
                    stage = p_stage.tile([128, 4, 4, 512], F32, tag="stage")
                    nc.vector.memset(stage[:, 0, 0, 0:4], 0.0)
                    st_eng = nc.sync if g == 0 else nc.scalar
                    for h in range(2):
                        st_eng.dma_start(
                            out_v[:, g * 4 + h * 2:g * 4 + (h + 1) * 2, D:4 * D],
                            stage[:, h * 2:(h + 1) * 2, 0:3, :])
                continue

            # context passthrough: data ready at batch start; issue early so
            # the DMA engines have work while this batch's compute runs.
            nc.sync.dma_start(out_v[:, :, 0:D], ctx_elem)

            # ---- query transposes: qt (plain q^T), qwt (q^T * Wcq) ----
            ps_q = ps_tp.tile([128, 512], F32R, tag="tp")
            for t in range(DT):
                nc.tensor.transpose(
                    ps_q[:, t * 128:(t + 1) * 128],
                    q_r[:, t * 128:(t + 1) * 128], ident_r[:])
            qt_sb = p_qt.tile([128, DT * 128], F32R, tag="qt")
            nc.scalar.copy(qt_sb[:], ps_q[:])
            qwt_sb = p_qt.tile([128, DT * 128], F32R, tag="qwt")
            for t in range(DT):
                nc.scalar.activation(
                    qwt_sb[:, t * 128:(t + 1) * 128],
                    ps_q[:, t * 128:(t + 1) * 128],
                    AF.Copy, scale=wpack[:, 2 * DT + t:2 * DT + t + 1])

            # ---- context transposes: ctxt[d-part][dt, c] (f32r) ----
            ctxt_sb = p_ctxt.tile([128, DT, C], F32R, tag="ctxt")
            for t in range(DT):
                for g in range(2):
                    ps_c = ps_tp.tile([128, 512], tp_dtype, tag="tp")
                    for i in range(4):
                        ct = g * 4 + i
                        tp_src = (ctx_f if cast_mode == "2" else ctx_r)
                        nc.tensor.transpose(
                            ps_c[:, i * 128:(i + 1) * 128],
                            tp_src[:, ct, t * 128:(t + 1) * 128], tp_ident[:])
                    eng = nc.scalar if t < 2 else nc.vector
                    if eng is nc.scalar:
                        eng.copy(ctxt_sb[:, t, g * 512:(g + 1) * 512], ps_c[:])
                    else:
                        eng.tensor_copy(ctxt_sb[:, t, g * 512:(g + 1) * 512],
                                        ps_c[:])

            # ---- q_sim column via PE (N=2, col 1 junk) ----
            ps_qs = ps_sim.tile([128, 512], F32, tag="sim")
            for t in range(DT):
                nc.tensor.matmul(
                    ps_qs[:, 0:2],
                    qt_sb[:, t * 128:(t + 1) * 128],
                    wpack_r[:, t:t + 2],
                    start=(t == 0), stop=(t == DT - 1))
            bias_col = p_small.tile([128, 1], F32, tag="biascol")
            nc.vector.tensor_scalar_add(bias_col[:], ps_qs[:, 0:1], bias_f)

            # ---- c_sim^T[1, c] = Wc^T @ ctxt (lhsT M=2, row 1 junk) ----
            csim_sb = p_csim.tile([1, C], F32R, tag="csim")
            for g in range(2):
                ps_cs = ps_tp.tile([2, 512], F32, tag="tp")
                for t in range(DT):
                    nc.tensor.matmul(
                        ps_cs[:],
                        wpack_r[:, DT + t:DT + t + 2],
                        ctxt_sb[:, t, g * 512:(g + 1) * 512],
                        start=(t == 0), stop=(t == DT - 1))
                nc.scalar.copy(csim_sb[:, g * 512:(g + 1) * 512],
                               ps_cs[0:1, :])

            # ---- sim^T = qwt^T @ ctxt + ones^T @ csim; E^T = exp(+qsim) ----
            et_sb = p_et.tile([128, C], F32R, tag="et")
            cs_parts = p_small.tile([128, 2], F32, tag="csparts")
            for g in range(2):
                ps_s = ps_sim.tile([128, 512], F32, tag="sim")
                for t in range(DT):
                    nc.tensor.matmul(
                        ps_s[:],
                        qwt_sb[:, t * 128:(t + 1) * 128],
                        ctxt_sb[:, t, g * 512:(g + 1) * 512],
                        start=(t == 0), stop=False)
                nc.tensor.matmul(
                    ps_s[:], ones_row[:],
                    csim_sb[:, g * 512:(g + 1) * 512],
                    start=False, stop=True)
                nc.scalar.activation(
                    et_sb[:, g * 512:(g + 1) * 512], ps_s[:],
                    AF.Exp, bias=bias_col[:],
                    accum_out=cs_parts[:, g:g + 1])

            cs_col = p_small.tile([128, 1], F32, tag="cscol")
            nc.vector.tensor_add(cs_col[:], cs_parts[:, 0:1], cs_parts[:, 1:2])
            rcs_col = p_small.tile([128, 1], F32, tag="rcscol")
            nc.vector.reciprocal(rcs_col[:], cs_col[:])

            # ---- E tiles [c-part, ct, q] via PE transpose (f32r) ----
            e_sb = p_e.tile([128, CT, 128], e_dtype, tag="e")
            for g in range(2):
                ps_e = ps_tp.tile([128, 512], F32R, tag="tp")
                for i in range(4):
                    ct = g * 4 + i
                    nc.tensor.transpose(
                        ps_e[:, i * 128:(i + 1) * 128],
                        et_sb[:, ct * 128:(ct + 1) * 128],
                        ident_r[:])
                nc.scalar.copy(
                    e_sb[:, g * 4:(g + 1) * 4, :].rearrange("p t q -> p (t q)"),
                    ps_e[:])

            # rowsums rs[c] = sum_q E[c,q] in one DVE reduce; rrs = 1/rs
            rs_sb = p_small.tile([128, CT], F32, tag="rs")
            nc.vector.tensor_reduce(rs_sb[:], e_sb[:], axis=AX.X, op=ALU.add)
            rrs_sb = p_small.tile([128, CT], F32, tag="rrs")
            nc.vector.reciprocal(rrs_sb[:], rs_sb[:])

            # ---- C2 = S2^T @ ctx = (E^T-sums) / cs   (f32r full rate) ----
            ps_c2 = ps_sim.tile([128, 512], F32, tag="sim")
            for ct in range(CT):
                nc.tensor.matmul(
                    ps_c2[:],
                    e_sb[:, ct, :],
                    (ctx_f if cast_mode == "2" else ctx_r)[:, ct, :],
                    start=(ct == 0), stop=(ct == CT - 1))
            c2_sb = p_c2.tile([128, D], F32R, tag="c2")
            nc.scalar.activation(c2_sb[:], ps_c2[:], AF.Copy, scale=rcs_col[:])

            # ---- per c-tile: A, B, ctx*A, ctx*B into staging; DMA out ----
            # stage quadrants per tile: [A | CA | CB | B]; only 0:3 stored.
            n_pool = 0
            for g in range(2):
                stage = p_stage.tile([128, 4, 4, 512], F32, tag="stage")
                for i in range(4):
                    ct = g * 4 + i
                    ps = ps_ab.tile([128, 1024], F32, tag="ab")
                    nc.tensor.matmul(
                        ps[:, 0:512],
                        et_sb[:, ct * 128:(ct + 1) * 128],
                        q_r[:], start=True, stop=True)
                    nc.tensor.matmul(
                        ps[:, 512:1024],
                        et_sb[:, ct * 128:(ct + 1) * 128],
                        c2_sb[:], start=True, stop=True)
                    # A = ps[:, 0:512]/rs on ACT; B = ps[:, 512:1024]/rs on DVE
                    nc.scalar.activation(
                        stage[:, i, 0, :], ps[:, 0:512], AF.Copy,
                        scale=rrs_sb[:, ct:ct + 1])
                    nc.vector.tensor_scalar_mul(
                        stage[:, i, 3, :], ps[:, 512:1024],
                        rrs_sb[:, ct:ct + 1])
                    # CA = ctx*A ; CB = ctx*B
                    ctx_ct = (ctx_f[:, ct, :] if cast_mode == "2"
                              else ctx_r[:, ct, :].bitcast(F32))
                    if n_pool < n_pool_muls:
                        n_pool += 1
                        nc.gpsimd.tensor_mul(
                            stage[:, i, 2, :], ctx_ct, stage[:, i, 3, :])
                    else:
                        nc.vector.tensor_mul(
                            stage[:, i, 2, :], ctx_ct, stage[:, i, 3, :])
                    nc.vector.tensor_mul(
                        stage[:, i, 1, :], ctx_ct, stage[:, i, 0, :])
                    if (i + 1) % st_gran == 0:
                        # store per st_gran c-tiles: earlier issue, smoother DMA
                        st_eng = nc.sync if g == 0 else nc.scalar
                        i0 = i + 1 - st_gran
                        st_eng.dma_start(
                            out_v[:, g * 4 + i0:g * 4 + i + 1, D:4 * D],
                            stage[:, i0:i + 1, 0:3, :])

    nc.compile()
    return nc


def _numpy_reference(context, query, c_mask, q_mask, Wq, Wc, Wcq, bias):
    """Exact fallback (matches reference.py) for non-all-ones masks."""
    NEG = -1e30
    q_sim = (query @ Wq[:, 0])[:, None, :]
    c_sim = (context @ Wc[:, 0])[:, :, None]
    cq_sim = np.einsum("bcd,bqd->bcq", context * Wcq, query)
    sim = q_sim + c_sim + cq_sim + bias
    qm = q_mask[:, None, :]
    cm = c_mask[:, :, None]
    q_logits = sim * qm + (1.0 - qm) * NEG
    c_logits = sim * cm + (1.0 - cm) * NEG

    def softmax(x, axis):
        x = x - x.max(axis=axis, keepdims=True)
        e = np.exp(x)
        return e / e.sum(axis=axis, keepdims=True)

    S1 = softmax(q_logits, -1)
    S2 = softmax(c_logits, 1)
    A = np.einsum("bcq,bqd->bcd", S1, query)
    B = np.einsum("bcq,bqd->bcd", S1, np.einsum("bkq,bkd->bqd", S2, context))
    return np.concatenate([context, A, context * A, context * B],
                          axis=2).astype(np.float32)


def make_in_maps(inputs) -> list:
    context = np.ascontiguousarray(np.asarray(inputs["context"], dtype=np.float32))
    query = np.ascontiguousarray(np.asarray(inputs["query"], dtype=np.float32))
    Wq = np.asarray(inputs["Wq"], dtype=np.float32)
    Wc = np.asarray(inputs["Wc"], dtype=np.float32)
    Wcq = np.asarray(inputs["Wcq"], dtype=np.float32)

    # pack the tiny weight vectors as [128, DT] columns (d = t*128 + p)
    def cols(w):
        return np.ascontiguousarray(w.reshape(DT, 128).T.astype(np.float32))

    wpack = np.concatenate(
        [cols(Wq[:, 0]), cols(Wc[:, 0]), cols(Wcq.reshape(-1))], axis=1)

    in_maps = []
    for i in range(N_CORES):
        in_maps.append({
            "context": np.ascontiguousarray(context[i * BPC:(i + 1) * BPC]),
            "query": np.ascontiguousarray(query[i * BPC:(i + 1) * BPC]),
            "wpack": wpack,
        })
    return in_maps


def kernel(**inputs) -> np.ndarray:
    c_mask = np.asarray(inputs["c_mask"], dtype=np.float32)
    q_mask = np.asarray(inputs["q_mask"], dtype=np.float32)
    bias = np.asarray(inputs["bias"], dtype=np.float32)

    if not (np.all(c_mask == 1.0) and np.all(q_mask == 1.0)):
        return _numpy_reference(
            np.asarray(inputs["context"], dtype=np.float32),
            np.asarray(inputs["query"], dtype=np.float32),
            c_mask, q_mask,
            np.asarray(inputs["Wq"], dtype=np.float32),
            np.asarray(inputs["Wc"], dtype=np.float32),
            np.asarray(inputs["Wcq"], dtype=np.float32),
            float(bias.reshape(-1)[0]))

    nc = build_program(float(bias.reshape(-1)[0]))
    in_maps = make_in_maps(inputs)
    try:
        res = run_bass_kernel_spmd(nc, in_maps, core_ids=list(range(N_CORES)))
    except Exception:
        # transient device wedge (e.g. NRT_EXEC_UNIT_UNRECOVERABLE): one retry
        res = run_bass_kernel_spmd(nc, in_maps, core_ids=list(range(N_CORES)))
    global last_results
    last_results = res
    out = np.concatenate([res.results[i]["out"] for i in range(N_CORES)], axis=0)
    return out


last_results = None
